# revision 8
# baseline (speedup 1.0000x reference)
"""CoaT factorized-attention block kernel for Trainium2, 8 NeuronCores.

Sharding: data-parallel over batch B=8 -> one batch element per core.

Per-core pipeline (attention-path tensors in head-aligned [96, *] layout):
  P1  (per 128-token chunk): k,v = x @ Wk/Wv (6 k-tiles of 128, PE);
      E = exp(k) (ACT); kv_aug[h] += E_h^T @ [v_h | 1] (PE, PSUM-resident
      accumulator, ones fused as 97th column); v^T via PE transpose into
      y-padded flat vt (+ element-shifted twin for k7 heads).
      DVE depthwise-conv row-bands for k7 heads are emitted as soon as
      their input rows are complete, overlapping pass 1.
  P2  kv[h] = kv_aug[:, :96] * scale / kv_aug[:, 96] + scale*bv (DVE).
  P3  cls column: q_cls(+bq), factor, proj (tiny).
  P4  (per 8-row group g of 448 cols): per head: q^T (PE, 6 k-tiles,
      +bq via ACT bias); factor (PE); conv: k3/k5 heads (+k7 late groups)
      as per-tap diagonal matmuls on PE, k7 early groups pre-computed on
      DVE; ev/att combine (DVE); proj (PE) + bias (ACT).

qkv bias handled exactly: k-bias is a softmax no-op; q-bias via ACT
per-partition bias on qt; v-bias folded into conv bias (host) and a
rank-1 kv correction (device).
Matmul inputs bf16 (fp32 PSUM accumulation); DVE conv accumulates bf16
two-level (per-dy-row partials) to bound rounding.
"""
import numpy as np
import ml_dtypes

B, N, C = 8, 3137, 768
NH, CH = 8, 96            # heads, channels per head
H = W = 56
HW = H * W                # 3136 == N - 1
NK = 6                    # contraction k-tiles of 128 over 768
HEAD_KS = [3, 3, 5, 5, 5, 7, 7, 7]
SCALE = CH ** -0.5
GROUPS = 7                # image-row groups of 8 rows = 448 cols each
GC = 8 * W                # 448
YPAD = 3                  # zero rows above/below image in vt
VROW = H + 2 * YPAD       # 62
VFLAT = VROW * W          # 3472
VOFF = YPAD * W           # 168: flat offset of image row 0

bf16 = ml_dtypes.bfloat16


def _head_taps(k):
    p = k // 2
    return [(0, 0)] + [(dy, dx) for dy in range(-p, p + 1)
                       for dx in range(-p, p + 1) if (dy, dx) != (0, 0)]


TAPS = [_head_taps(k) for k in HEAD_KS]
TAP_OFF = np.cumsum([0] + [len(t) for t in TAPS]).tolist()
NTAP = TAP_OFF[-1]  # 240

# conv engine split: for k7 heads (5,6,7), groups 0..DVE_G[h]-1 run on DVE
# (emitted as 16/8-row bands during pass 1), the rest on PE.
DVE_G = {5: 6, 6: 5, 7: 5}
K7 = [5, 6, 7]

_PROG = None


def _dve_bands(h):
    """Row-bands (r0, r1) covering groups 0..DVE_G[h]-1, 16 rows each."""
    rows = 8 * DVE_G[h]
    bands = []
    r = 0
    while r < rows:
        bands.append((r, min(r + 16, rows)))
        r += 16
    return bands


def _band_ready_chunk(r1):
    """Index (into the 24 image chunks) after which rows < r1+3 exist."""
    need = min(HW, (r1 + YPAD) * W)  # tokens needed (image-indexed)
    return min(23, (need + 127) // 128 - 1)


def _build_program():
    import concourse.bass as bass
    from concourse import bacc
    import concourse.mybir as mybir
    import concourse.tile as tile
    from concourse.masks import make_identity
    from contextlib import ExitStack

    fp32 = mybir.dt.float32
    bf = mybir.dt.bfloat16
    AL = mybir.AluOpType
    ACT = mybir.ActivationFunctionType

    nc = bacc.Bacc("TRN2", target_bir_lowering=False, debug=False, num_devices=8)

    xT_d = nc.dram_tensor("xT", [NK * 128, N], bf, kind="ExternalInput")
    wq_d = nc.dram_tensor("wq", [NK * 128, C], bf, kind="ExternalInput")
    wk_d = nc.dram_tensor("wk", [NK * 128, C], bf, kind="ExternalInput")
    wv_d = nc.dram_tensor("wv", [NK * 128, C], bf, kind="ExternalInput")
    pw_d = nc.dram_tensor("pw96", [CH, NH, C], bf, kind="ExternalInput")
    pb_d = nc.dram_tensor("pb2", [128, 6], fp32, kind="ExternalInput")
    cb_d = nc.dram_tensor("cb96", [CH, NH], fp32, kind="ExternalInput")
    bq_d = nc.dram_tensor("bq96", [CH, NH], fp32, kind="ExternalInput")
    bvs_d = nc.dram_tensor("bvs", [CH, NH, CH], bf, kind="ExternalInput")
    dg_d = nc.dram_tensor("dg", [CH, NTAP, CH], bf, kind="ExternalInput")
    cw_d = nc.dram_tensor("cw", [CH, NH, 49], fp32, kind="ExternalInput")
    out_d = nc.dram_tensor("outT", [C, N], fp32, kind="ExternalOutput")

    xT_r = xT_d[:, :].rearrange("(t p) n -> p t n", p=128)  # [128, 6, N]
    wq_r = wq_d[:, :].rearrange("(t p) c -> p t c", p=128)
    wk_r = wk_d[:, :].rearrange("(t p) c -> p t c", p=128)
    wv_r = wv_d[:, :].rearrange("(t p) c -> p t c", p=128)

    # token chunks for pass 1: cls + 24 full + tail(64); image chunk i
    # covers image tokens [128i, 128i+sz).
    chunks = [(0, 1)] + [(1 + 128 * t, 128) for t in range(24)] + [(3073, 64)]

    with tile.TileContext(nc) as tc, ExitStack() as ctx:
        const = ctx.enter_context(tc.tile_pool(name="const", bufs=1))
        wq_sb = const.tile([128, NK, C], bf)
        nc.sync.dma_start(wq_sb, wq_r)
        pw_sb = const.tile([CH, NH, C], bf)
        nc.sync.dma_start(pw_sb, pw_d[:, :, :])
        pb_sb = const.tile([128, 6], fp32)
        nc.sync.dma_start(pb_sb, pb_d[:, :])
        cb_sb = const.tile([CH, NH], fp32)
        nc.sync.dma_start(cb_sb, cb_d[:, :])
        bq_sb = const.tile([CH, NH], fp32)
        nc.sync.dma_start(bq_sb, bq_d[:, :])
        bvs_sb = const.tile([CH, NH, CH], bf)
        nc.sync.dma_start(bvs_sb, bvs_d[:, :, :])
        dg_sb = const.tile([CH, NTAP, CH], bf)
        nc.sync.dma_start(dg_sb, dg_d[:, :, :])
        cw_sb = const.tile([CH, NH, 49], fp32)
        nc.sync.dma_start(cw_sb, cw_d[:, :, :])
        ident = const.tile([128, 128], bf)
        make_identity(nc, ident)

        # y-padded flat v^T per head; halo rows zero
        vt = const.tile([CH, NH, VFLAT], bf)
        for h in range(NH):
            nc.vector.memset(vt[:, h, 0:VOFF], 0.0)
            nc.vector.memset(vt[:, h, VOFF + HW:VFLAT], 0.0)

        kv_sb = const.tile([CH, NH, CH], bf)
        r_sb = const.tile([CH, NH], fp32)
        # DVE conv accumulators: one per (k7 head, band)
        acc_t = {(h, bi): const.tile([CH, 16 * W], bf, name=f"acc{h}_{bi}")
                 for h in K7 for bi in range(len(_dve_bands(h)))}

        conv_pool = ctx.enter_context(tc.tile_pool(name="cvtmp", bufs=2))

        # DVE band strips: rows [r0-3, r1+3) re-laid with stride 62 and zero
        # x-halos so every tap is a full-width, 4B-aligned op. Two copies:
        # image col c at offset 4+c (even-dx taps) and 3+c (odd-dx taps).
        SROWS = 22  # max 16 + 2*YPAD

        def emit_dve_conv(h, bi):
            """Depthwise conv rows [r0,r1) for head h on DVE, two-level bf16."""
            r0, r1 = _dve_bands(h)[bi]
            nr = r1 - r0
            ns = nr + 2 * YPAD
            acc = acc_t[(h, bi)][:, 0:nr * W]
            alo_t = conv_pool.tile([CH, 16 * W], bf, tag="alo", name="alo")
            tmp_t = conv_pool.tile([CH, 16 * W], bf, tag="tmp", name="tmp")
            se_t = conv_pool.tile([CH, SROWS * 62], bf, tag="se", name="se")
            so_t = conv_pool.tile([CH, SROWS * 62], bf, tag="so", name="so")
            alo = alo_t[:, 0:nr * W]
            tmp = tmp_t[:, 0:nr * W]
            se3 = se_t[:, 0:ns * 62].rearrange("p (y x) -> p y x", x=62)
            so3 = so_t[:, 0:ns * 62].rearrange("p (y x) -> p y x", x=62)
            # build strips: strip row i = image row r0-3+i (halo rows zero)
            yy0, yy1 = max(0, r0 - YPAD), min(H, r1 + YPAD)
            i0, i1 = yy0 - (r0 - YPAD), yy1 - (r0 - YPAD)
            vh3 = vt[:, h, :].rearrange("p (y x) -> p y x", x=W)
            nc.vector.memset(se_t[:, 0:ns * 62], 0.0)
            nc.vector.tensor_copy(se3[:, i0:i1, 4:4 + W],
                                  vh3[:, yy0 + YPAD:yy1 + YPAD, :])
            nc.vector.memset(so_t[:, 0:ns * 62], 0.0)
            nc.vector.tensor_copy(so3[:, i0:i1, 3:3 + W],
                                  vh3[:, yy0 + YPAD:yy1 + YPAD, :])
            k = HEAD_KS[h]
            p = k // 2
            t_of = {d: t for t, d in enumerate(TAPS[h])}
            for ri, dy in enumerate(range(-p, p + 1)):
                # strip rows for output rows [r0, r1) shifted by dy
                slo = YPAD + dy
                w0 = cw_sb[:, h, t_of[(dy, 0)]:t_of[(dy, 0)] + 1]
                nc.vector.tensor_scalar(
                    alo, se3[:, slo:slo + nr, 4:4 + W], w0, None, op0=AL.mult)
                for dx in list(range(-p, 0)) + list(range(1, p + 1)):
                    wt = cw_sb[:, h, t_of[(dy, dx)]:t_of[(dy, dx)] + 1]
                    if dx % 2 == 0:
                        src = se3[:, slo:slo + nr, 4 + dx:4 + dx + W]
                    else:
                        src = so3[:, slo:slo + nr, 3 + dx:3 + dx + W]
                    nc.vector.tensor_scalar(tmp, src, wt, None, op0=AL.mult)
                    nc.vector.tensor_tensor(alo, tmp, alo, op=AL.add)
                if ri == 0:
                    nc.vector.tensor_copy(acc, alo)
                else:
                    nc.vector.tensor_tensor(acc, alo, acc, op=AL.add)

        # ---------------- pass 1: k, v, E, kv accumulation, v transposes
        with tc.tile_pool(name="p1w", bufs=1) as p1w, \
             tc.tile_pool(name="p1roll", bufs=3) as p1roll, \
             tc.tile_pool(name="p1ps", bufs=2, space="PSUM") as p1ps, \
             tc.tile_pool(name="kvps", bufs=1, space="PSUM") as kvps:
            wk_sb = p1w.tile([128, NK, C], bf)
            nc.sync.dma_start(wk_sb, wk_r)
            wv_sb = p1w.tile([128, NK, C], bf)
            nc.sync.dma_start(wv_sb, wv_r)

            kv_ps = [kvps.tile([CH, 4, CH + 1], fp32, tag=f"kv{i}",
                               name=f"kvps{i}") for i in range(2)]

            pend = {}  # image-chunk idx -> [(h, bi), ...] to emit after
            for h in K7:
                for bi, (r0, r1) in enumerate(_dve_bands(h)):
                    pend.setdefault(_band_ready_chunk(r1), []).append((h, bi))

            for ci, (n0, sz) in enumerate(chunks):
                first, last = ci == 0, ci == len(chunks) - 1
                xh = p1roll.tile([128, NK, 128], bf, tag="xh")
                nc.sync.dma_start(xh[:, :, :sz], xT_r[:, :, n0:n0 + sz])

                v_sb = p1roll.tile([128, NH, CH + 1], bf, tag="v")
                nc.vector.memset(v_sb[:, :, CH:CH + 1], 1.0)
                e_sb = p1roll.tile([128, NH, CH], bf, tag="e")
                for half in range(2):
                    c0 = half * 384
                    pv = p1ps.tile([128, 384], fp32, tag="pv")
                    pk = p1ps.tile([128, 384], fp32, tag="pk")
                    for k in range(NK):
                        nc.tensor.matmul(pv[:sz], xh[:, k, :sz],
                                         wv_sb[:, k, c0:c0 + 384],
                                         start=(k == 0), stop=(k == NK - 1))
                    for k in range(NK):
                        nc.tensor.matmul(pk[:sz], xh[:, k, :sz],
                                         wk_sb[:, k, c0:c0 + 384],
                                         start=(k == 0), stop=(k == NK - 1))
                    pv3 = pv[:sz].rearrange("p (a b) -> p a b", a=4)
                    pk3 = pk[:sz].rearrange("p (a b) -> p a b", a=4)
                    nc.scalar.copy(v_sb[:sz, 4 * half:4 * half + 4, 0:CH], pv3)
                    nc.scalar.activation(e_sb[:sz, 4 * half:4 * half + 4, :],
                                         pk3, ACT.Exp)

                for h in range(NH):
                    kvp = kv_ps[h // 4]
                    nc.tensor.matmul(kvp[:, h % 4, :],
                                     e_sb[:sz, h, :],
                                     v_sb[:sz, h, :],
                                     start=first, stop=last,
                                     skip_group_check=True)

                if not first:  # transpose image tokens into vt (+ twin)
                    j0 = n0 - 1
                    for h in range(NH):
                        tp = p1ps.tile([CH, 128], bf, tag="tp")
                        nc.tensor.transpose(tp[:, :sz], v_sb[:sz, h, 0:CH],
                                            ident[:sz, :sz])
                        dst = vt[:, h, VOFF + j0:VOFF + j0 + sz]
                        if h % 2 == 0:
                            nc.vector.tensor_copy(dst, tp[:, :sz])
                        else:
                            nc.scalar.copy(dst, tp[:, :sz])

                if not first and not last:
                    for (h, bi) in pend.pop(ci - 1, []):
                        emit_dve_conv(h, bi)

            for lst in pend.values():
                for (h, bi) in lst:
                    emit_dve_conv(h, bi)

            # finalize kv: kv = kv_aug[:, :96] * (scale/den) + scale*bv
            for h in range(NH):
                kvp = kv_ps[h // 4]
                nc.vector.reciprocal(r_sb[:, h:h + 1], kvp[:, h % 4, CH:CH + 1])
                nc.vector.tensor_scalar(kv_sb[:, h, :], kvp[:, h % 4, 0:CH],
                                        r_sb[:, h:h + 1], float(SCALE),
                                        op0=AL.mult, op1=AL.mult)
                nc.vector.tensor_tensor(kv_sb[:, h, :], bvs_sb[:, h, :],
                                        kv_sb[:, h, :], op=AL.add)

        # ---------------- pass 2: per-group q + factor + conv + att + proj
        with tc.tile_pool(name="p4roll", bufs=4) as p4roll, \
             tc.tile_pool(name="p4att", bufs=2) as p4att, \
             tc.tile_pool(name="p4xg", bufs=2) as p4xg, \
             tc.tile_pool(name="p4ps", bufs=2, space="PSUM") as p4ps:

            # cls column (token 0): factor_att only, crpe = 0
            xc = p4xg.tile([128, NK, GC], bf, tag="xg")
            nc.sync.dma_start(xc[:, :, 0:1], xT_r[:, :, 0:1])
            pqc = p4ps.tile([CH, NH], fp32, tag="pq")
            for h in range(NH):
                for k in range(NK):
                    nc.tensor.matmul(pqc[:, h:h + 1],
                                     wq_sb[:, k, h * CH:(h + 1) * CH],
                                     xc[:, k, 0:1], start=(k == 0),
                                     stop=(k == NK - 1), skip_group_check=True)
            qtc = p4roll.tile([CH, NH], bf, tag="qtc")
            nc.scalar.copy(qtc, pqc)
            nc.vector.tensor_tensor(qtc, bq_sb, qtc, op=AL.add)
            pfc = p4ps.tile([CH, NH], fp32, tag="pf")
            for h in range(NH):
                nc.tensor.matmul(pfc[:, h:h + 1], kv_sb[:, h, :],
                                 qtc[:, h:h + 1], start=True, stop=True,
                                 skip_group_check=True)
            atc = p4roll.tile([CH, NH], bf, tag="atc")
            nc.scalar.copy(atc, pfc)
            poc = p4ps.tile([128, GC], fp32, tag="po")
            for e in range(6):
                for h in range(NH):
                    nc.tensor.matmul(poc[:, e:e + 1],
                                     pw_sb[:, h, e * 128:(e + 1) * 128],
                                     atc[:, h:h + 1], start=(h == 0),
                                     stop=(h == NH - 1), skip_group_check=True)
            ocs = p4roll.tile([128, 6], fp32, tag="ocs")
            for e in range(6):
                nc.scalar.activation(ocs[:, e:e + 1], poc[:, e:e + 1],
                                     ACT.Identity, bias=pb_sb[:, e:e + 1])
                nc.sync.dma_start(out_d[e * 128:(e + 1) * 128, 0:1],
                                  ocs[:, e:e + 1])

            # main grouped loop
            vt3 = [vt[:, h, :].rearrange("p (y x) -> p y x", x=W)
                   for h in range(NH)]
            for g in range(GROUPS):
                gy0 = g * 8
                n0 = 1 + g * GC
                xg = p4xg.tile([128, NK, GC], bf, tag="xg")
                nc.sync.dma_start(xg, xT_r[:, :, n0:n0 + GC])

                att = p4att.tile([CH, NH, GC], bf, tag="att")
                for h in range(NH):
                    on_dve = h in K7 and g < DVE_G[h]

                    # q^T chunk for this head (+ q bias)
                    pq = p4ps.tile([CH, GC], fp32, tag="pq")
                    for k in range(NK):
                        nc.tensor.matmul(pq, wq_sb[:, k, h * CH:(h + 1) * CH],
                                         xg[:, k, :], start=(k == 0),
                                         stop=(k == NK - 1))
                    qt = p4roll.tile([CH, GC], bf, tag="qt")
                    nc.scalar.activation(qt, pq, ACT.Identity,
                                         bias=bq_sb[:, h:h + 1])

                    # factor_att
                    pf = p4ps.tile([CH, GC], fp32, tag="pf")
                    nc.tensor.matmul(pf, kv_sb[:, h, :], qt,
                                     start=True, stop=True)

                    ev = p4roll.tile([CH, GC], bf, tag="ev")
                    if on_dve:
                        bi, half = g // 2, g % 2
                        acc = acc_t[(h, bi)][:, half * GC:half * GC + GC]
                        # ev = (acc + cb) * qt
                        nc.vector.scalar_tensor_tensor(
                            ev, acc, cb_sb[:, h:h + 1], qt,
                            op0=AL.add, op1=AL.mult)
                    else:
                        # conv: per-tap diagonal matmuls accumulating in psum
                        pcv = p4ps.tile([CH, 8, W], fp32, tag="pcv")
                        t_base = TAP_OFF[h]
                        for t, (dy, dx) in enumerate(TAPS[h]):
                            x0, x1 = max(0, -dx), W - max(0, dx)
                            ylo, yhi = gy0 + dy + YPAD, gy0 + 8 + dy + YPAD
                            nc.tensor.matmul(
                                pcv[:, 0:8, x0:x1], dg_sb[:, t_base + t, :],
                                vt3[h][:, ylo:yhi, x0 + dx:x1 + dx],
                                start=(t == 0), stop=(t == len(TAPS[h]) - 1),
                                skip_group_check=True)
                        cv = p4roll.tile([CH, GC], bf, tag="cv")
                        nc.scalar.activation(cv,
                                             pcv.rearrange("p a b -> p (a b)"),
                                             ACT.Identity,
                                             bias=cb_sb[:, h:h + 1])
                        nc.vector.tensor_tensor(ev, qt, cv, op=AL.mult)
                    # att = ev + factor (reads pf straight from PSUM)
                    nc.vector.scalar_tensor_tensor(att[:, h, :], ev, 1.0, pf,
                                                   op0=AL.mult, op1=AL.add)

                # proj for this group of columns
                for e in range(6):
                    po = p4ps.tile([128, GC], fp32, tag="po")
                    for h in range(NH):
                        nc.tensor.matmul(po, pw_sb[:, h, e * 128:(e + 1) * 128],
                                         att[:, h, :], start=(h == 0),
                                         stop=(h == NH - 1))
                    osb = p4roll.tile([128, GC], fp32, tag="osb")
                    nc.scalar.activation(osb, po, ACT.Identity,
                                         bias=pb_sb[:, e:e + 1])
                    nc.sync.dma_start(out_d[e * 128:(e + 1) * 128, n0:n0 + GC],
                                      osb)

    nc.compile()
    return nc


def _get_program():
    global _PROG
    if _PROG is None:
        _PROG = _build_program()
    return _PROG


def _host_prep(x, qkv_w, qkv_b, proj_w, proj_b,
               conv3_w, conv3_b, conv5_w, conv5_b, conv7_w, conv7_b):
    """Build per-core input dicts (shared weight tensors prepped once)."""
    qkv_w = np.asarray(qkv_w, np.float32)
    qkv_b = np.asarray(qkv_b, np.float32)
    proj_w = np.asarray(proj_w, np.float32)
    proj_b = np.asarray(proj_b, np.float32)

    def wslab(sl):
        return np.ascontiguousarray(qkv_w[sl].T).astype(bf16)

    wq = wslab(slice(0, C))
    wk = wslab(slice(C, 2 * C))
    wv = wslab(slice(2 * C, 3 * C))
    bq96 = np.ascontiguousarray(qkv_b[0:C].reshape(NH, CH).T).astype(np.float32)
    bv = qkv_b[2 * C:3 * C].reshape(NH, CH)  # [h, c]

    pw96 = np.ascontiguousarray(
        proj_w.T.reshape(NH, CH, C).transpose(1, 0, 2)).astype(bf16)
    pb2 = np.ascontiguousarray(proj_b.reshape(6, 128).T).astype(np.float32)

    conv_w = [np.asarray(w, np.float32) for w in (conv3_w, conv5_w, conv7_w)]
    conv_b = [np.asarray(b, np.float32) for b in (conv3_b, conv5_b, conv7_b)]
    grp_of_head = [0, 0, 1, 1, 1, 2, 2, 2]
    head_in_grp = [0, 1, 0, 1, 2, 0, 1, 2]

    cb96 = np.zeros((CH, NH), np.float32)
    dg = np.zeros((CH, NTAP, CH), np.float32)
    cw = np.zeros((CH, NH, 49), np.float32)
    bvs = np.zeros((CH, NH, CH), np.float32)
    for h in range(NH):
        k = HEAD_KS[h]
        p = k // 2
        gidx, hig = grp_of_head[h], head_in_grp[h]
        wfull = conv_w[gidx][hig * CH:(hig + 1) * CH, 0]  # [96, k, k]
        # conv bias + folded v-bias: conv(v0+bv) = conv(v0) + sum(w)*bv
        cb96[:, h] = (conv_b[gidx][hig * CH:(hig + 1) * CH]
                      + wfull.sum(axis=(1, 2)) * bv[h])
        bvs[:, h, :] = SCALE * bv[h][None, :]
        for t, (dy, dx) in enumerate(TAPS[h]):
            np.fill_diagonal(dg[:, TAP_OFF[h] + t, :], wfull[:, dy + p, dx + p])
            cw[:, h, t] = wfull[:, dy + p, dx + p]
    dg = dg.astype(bf16)

    shared = {"wq": wq, "wk": wk, "wv": wv, "pw96": pw96, "pb2": pb2,
              "cb96": cb96, "bq96": bq96, "bvs": bvs.astype(bf16),
              "dg": dg, "cw": cw}

    x = np.asarray(x, np.float32)
    in_maps = []
    for b in range(B):
        m = dict(shared)
        m["xT"] = np.ascontiguousarray(x[b].T).astype(bf16)
        in_maps.append(m)
    return in_maps


def kernel(x, qkv_w, qkv_b, proj_w, proj_b,
           conv3_w, conv3_b, conv5_w, conv5_b, conv7_w, conv7_b, H, W,
           _trace=False):
    assert int(H) == 56 and int(W) == 56
    x = np.asarray(x)
    assert x.shape == (B, N, C)

    from concourse.bass_utils import run_bass_kernel_spmd
    nc = _get_program()
    in_maps = _host_prep(x, qkv_w, qkv_b, proj_w, proj_b,
                         conv3_w, conv3_b, conv5_w, conv5_b, conv7_w, conv7_b)
    res = run_bass_kernel_spmd(nc, in_maps, core_ids=list(range(B)), trace=_trace)
    out = np.stack([res.results[b]["outT"].T for b in range(B)])
    if _trace:
        kernel._last_results = res
    return out.astype(np.float32)


# revision 16
# speedup vs baseline: 1.0002x; 1.0002x over previous
"""CoaT factorized-attention block kernel for Trainium2, 8 NeuronCores.

Sharding: data-parallel over batch B=8 -> one batch element per core.

Per-core pipeline (attention-path tensors in head-aligned [96, *] layout):
  P1  (per 128-token chunk): k,v = x @ Wk/Wv (6 k-tiles of 128, PE);
      E = exp(k) (ACT); kv_aug[h] += E_h^T @ [v_h | 1] (PE, PSUM-resident
      accumulator, ones fused as 97th column); v^T via PE transpose into
      y-padded flat vt (+ element-shifted twin for k7 heads).
      DVE depthwise-conv row-bands for k7 heads are emitted as soon as
      their input rows are complete, overlapping pass 1.
  P2  kv[h] = kv_aug[:, :96] * scale / kv_aug[:, 96] + scale*bv (DVE).
  P3  cls column: q_cls(+bq), factor, proj (tiny).
  P4  (per 8-row group g of 448 cols): per head: q^T (PE, 6 k-tiles,
      +bq via ACT bias); factor (PE); conv: k3/k5 heads (+k7 late groups)
      as per-tap diagonal matmuls on PE, k7 early groups pre-computed on
      DVE; ev/att combine (DVE); proj (PE) + bias (ACT).

qkv bias handled exactly: k-bias is a softmax no-op; q-bias via ACT
per-partition bias on qt; v-bias folded into conv bias (host) and a
rank-1 kv correction (device).
Matmul inputs bf16 (fp32 PSUM accumulation); DVE conv accumulates bf16
two-level (per-dy-row partials) to bound rounding.
"""
import numpy as np
import ml_dtypes

B, N, C = 8, 3137, 768
NH, CH = 8, 96            # heads, channels per head
H = W = 56
HW = H * W                # 3136 == N - 1
NK = 6                    # contraction k-tiles of 128 over 768
HEAD_KS = [3, 3, 5, 5, 5, 7, 7, 7]
SCALE = CH ** -0.5
GROUPS = 7                # image-row groups of 8 rows = 448 cols each
GC = 8 * W                # 448
YPAD = 3                  # zero rows above/below image in vt
VROW = H + 2 * YPAD       # 62
VFLAT = VROW * W          # 3472
VOFF = YPAD * W           # 168: flat offset of image row 0

bf16 = ml_dtypes.bfloat16


def _head_taps(k):
    p = k // 2
    return [(0, 0)] + [(dy, dx) for dy in range(-p, p + 1)
                       for dx in range(-p, p + 1) if (dy, dx) != (0, 0)]


TAPS = [_head_taps(k) for k in HEAD_KS]
TAP_OFF = np.cumsum([0] + [len(t) for t in TAPS]).tolist()
NTAP = TAP_OFF[-1]  # 240

# conv engine split: for k7 heads (5,6,7), groups 0..DVE_G[h]-1 run on DVE
# (emitted as 16/8-row bands, pumped incrementally), the rest on PE.
DVE_G = {5: 6, 6: 5, 7: 4}
K7 = [5, 6, 7]

_PROG = None


def _dve_bands(h):
    """Row-bands (r0, r1) covering groups 0..DVE_G[h]-1, 16 rows each."""
    rows = 8 * DVE_G[h]
    bands = []
    r = 0
    while r < rows:
        bands.append((r, min(r + 16, rows)))
        r += 16
    return bands


def _band_ready_chunk(r1):
    """Index (into the 24 image chunks) after which rows < r1+3 exist."""
    need = min(HW, (r1 + YPAD) * W)  # tokens needed (image-indexed)
    return min(23, (need + 127) // 128 - 1)


def _build_program():
    import concourse.bass as bass
    from concourse import bacc
    import concourse.mybir as mybir
    import concourse.tile as tile
    from concourse.masks import make_identity
    from contextlib import ExitStack

    fp32 = mybir.dt.float32
    bf = mybir.dt.bfloat16
    AL = mybir.AluOpType
    ACT = mybir.ActivationFunctionType

    nc = bacc.Bacc("TRN2", target_bir_lowering=False, debug=False, num_devices=8)

    xT_d = nc.dram_tensor("xT", [NK * 128, N], bf, kind="ExternalInput")
    wq_d = nc.dram_tensor("wq", [NK * 128, C], bf, kind="ExternalInput")
    wk_d = nc.dram_tensor("wk", [NK * 128, C], bf, kind="ExternalInput")
    wv_d = nc.dram_tensor("wv", [NK * 128, C], bf, kind="ExternalInput")
    pw_d = nc.dram_tensor("pw96", [CH, NH, C], bf, kind="ExternalInput")
    pb_d = nc.dram_tensor("pb2", [128, 6], fp32, kind="ExternalInput")
    cb_d = nc.dram_tensor("cb96", [CH, NH], fp32, kind="ExternalInput")
    bq_d = nc.dram_tensor("bq96", [CH, NH], fp32, kind="ExternalInput")
    bvs_d = nc.dram_tensor("bvs", [CH, NH, CH], bf, kind="ExternalInput")
    dg_d = nc.dram_tensor("dg", [CH, NTAP, CH], bf, kind="ExternalInput")
    cw_d = nc.dram_tensor("cw", [CH, NH, 49], fp32, kind="ExternalInput")
    out_d = nc.dram_tensor("outT", [C, N], fp32, kind="ExternalOutput")

    xT_r = xT_d[:, :].rearrange("(t p) n -> p t n", p=128)  # [128, 6, N]
    wq_r = wq_d[:, :].rearrange("(t p) c -> p t c", p=128)
    wk_r = wk_d[:, :].rearrange("(t p) c -> p t c", p=128)
    wv_r = wv_d[:, :].rearrange("(t p) c -> p t c", p=128)

    # token chunks for pass 1: cls + 24 full + tail(64); image chunk i
    # covers image tokens [128i, 128i+sz).
    chunks = [(0, 1)] + [(1 + 128 * t, 128) for t in range(24)] + [(3073, 64)]

    with tile.TileContext(nc) as tc, ExitStack() as ctx:
        const = ctx.enter_context(tc.tile_pool(name="const", bufs=1))
        wq_sb = const.tile([128, NK, C], bf)
        nc.sync.dma_start(wq_sb, wq_r)
        pw_sb = const.tile([CH, NH, C], bf)
        nc.sync.dma_start(pw_sb, pw_d[:, :, :])
        pb_sb = const.tile([128, 6], fp32)
        nc.sync.dma_start(pb_sb, pb_d[:, :])
        cb_sb = const.tile([CH, NH], fp32)
        nc.sync.dma_start(cb_sb, cb_d[:, :])
        bq_sb = const.tile([CH, NH], fp32)
        nc.sync.dma_start(bq_sb, bq_d[:, :])
        bvs_sb = const.tile([CH, NH, CH], bf)
        nc.sync.dma_start(bvs_sb, bvs_d[:, :, :])
        dg_sb = const.tile([CH, NTAP, CH], bf)
        nc.sync.dma_start(dg_sb, dg_d[:, :, :])
        cw_sb = const.tile([CH, NH, 49], fp32)
        nc.sync.dma_start(cw_sb, cw_d[:, :, :])
        ident = const.tile([128, 128], bf)
        make_identity(nc, ident)

        # y-padded flat v^T per head; halo rows zero
        vt = const.tile([CH, NH, VFLAT], bf)
        for h in range(NH):
            nc.vector.memset(vt[:, h, 0:VOFF], 0.0)
            nc.vector.memset(vt[:, h, VOFF + HW:VFLAT], 0.0)

        kv_sb = const.tile([CH, NH, CH], bf)
        r_sb = const.tile([CH, NH], fp32)
        # DVE conv accumulators: one per (k7 head, band)
        acc_t = {(h, bi): const.tile([CH, 16 * W], bf, name=f"acc{h}_{bi}")
                 for h in K7 for bi in range(len(_dve_bands(h)))}

        conv_pool = ctx.enter_context(tc.tile_pool(name="cvtmp", bufs=2))

        # DVE band strips: rows [r0-3, r1+3) re-laid with stride 62 and zero
        # x-halos so every tap is a full-width, 4B-aligned op. Two copies:
        # image col c at offset 4+c (even-dx taps) and 3+c (odd-dx taps).
        SROWS = 22  # max 16 + 2*YPAD

        def gen_dve_conv(h, bi):
            """Depthwise conv rows [r0,r1) for head h on DVE, two-level bf16.
            Generator: yields after each dy-row so emission can be pumped in
            small pieces between other engines' work."""
            r0, r1 = _dve_bands(h)[bi]
            nr = r1 - r0
            ns = nr + 2 * YPAD
            acc = acc_t[(h, bi)][:, 0:nr * W]
            alo_t = conv_pool.tile([CH, 16 * W], bf, tag="alo", name="alo")
            tmp_t = conv_pool.tile([CH, 16 * W], bf, tag="tmp", name="tmp")
            se_t = conv_pool.tile([CH, SROWS * 62], bf, tag="se", name="se")
            so_t = conv_pool.tile([CH, SROWS * 62], bf, tag="so", name="so")
            alo = alo_t[:, 0:nr * W]
            tmp = tmp_t[:, 0:nr * W]
            se3 = se_t[:, 0:ns * 62].rearrange("p (y x) -> p y x", x=62)
            so3 = so_t[:, 0:ns * 62].rearrange("p (y x) -> p y x", x=62)
            # build strips: strip row i = image row r0-3+i (halo rows zero)
            yy0, yy1 = max(0, r0 - YPAD), min(H, r1 + YPAD)
            i0, i1 = yy0 - (r0 - YPAD), yy1 - (r0 - YPAD)
            vh3 = vt[:, h, :].rearrange("p (y x) -> p y x", x=W)
            nc.vector.memset(se_t[:, 0:ns * 62], 0.0)
            nc.vector.tensor_copy(se3[:, i0:i1, 4:4 + W],
                                  vh3[:, yy0 + YPAD:yy1 + YPAD, :])
            nc.vector.memset(so_t[:, 0:ns * 62], 0.0)
            nc.vector.tensor_copy(so3[:, i0:i1, 3:3 + W],
                                  vh3[:, yy0 + YPAD:yy1 + YPAD, :])
            yield
            k = HEAD_KS[h]
            p = k // 2
            t_of = {d: t for t, d in enumerate(TAPS[h])}
            for ri, dy in enumerate(range(-p, p + 1)):
                # strip rows for output rows [r0, r1) shifted by dy
                slo = YPAD + dy
                w0 = cw_sb[:, h, t_of[(dy, 0)]:t_of[(dy, 0)] + 1]
                nc.vector.tensor_scalar(
                    alo, se3[:, slo:slo + nr, 4:4 + W], w0, None, op0=AL.mult)
                for dx in list(range(-p, 0)) + list(range(1, p + 1)):
                    wt = cw_sb[:, h, t_of[(dy, dx)]:t_of[(dy, dx)] + 1]
                    if dx % 2 == 0:
                        src = se3[:, slo:slo + nr, 4 + dx:4 + dx + W]
                    else:
                        src = so3[:, slo:slo + nr, 3 + dx:3 + dx + W]
                    nc.vector.tensor_scalar(tmp, src, wt, None, op0=AL.mult)
                    nc.vector.tensor_tensor(alo, tmp, alo, op=AL.add)
                if ri == 0:
                    nc.vector.tensor_copy(acc, alo)
                else:
                    nc.vector.tensor_tensor(acc, alo, acc, op=AL.add)
                yield

        # conv generators, in consumption order, gated by input readiness
        conv_q = []  # [h, bi, ready_chunk, gen-or-None]
        for bi in range(3):
            for h in K7:
                if bi < len(_dve_bands(h)):
                    r0, r1 = _dve_bands(h)[bi]
                    conv_q.append([h, bi, _band_ready_chunk(r1), None])
        conv_q.sort(key=lambda e: (e[1], e[0]))

        def pump_conv(steps, img_chunk=None):
            """Advance pending conv generators by `steps` yields. During
            pass 1 (img_chunk set) only generators whose rows exist run."""
            while steps > 0 and conv_q:
                e = conv_q[0]
                if img_chunk is not None and img_chunk < e[2]:
                    return
                if e[3] is None:
                    e[3] = gen_dve_conv(e[0], e[1])
                try:
                    next(e[3])
                    steps -= 1
                except StopIteration:
                    conv_q.pop(0)

        def drain_conv(bi_max):
            while conv_q and conv_q[0][1] <= bi_max:
                e = conv_q[0]
                if e[3] is None:
                    e[3] = gen_dve_conv(e[0], e[1])
                for _ in e[3]:
                    pass
                conv_q.pop(0)

        # ---------------- pass 1: k, v, E, kv accumulation, v transposes
        with tc.tile_pool(name="p1w", bufs=1) as p1w, \
             tc.tile_pool(name="p1roll", bufs=3) as p1roll, \
             tc.tile_pool(name="p1ps", bufs=2, space="PSUM") as p1ps, \
             tc.tile_pool(name="kvps", bufs=1, space="PSUM") as kvps:
            wk_sb = p1w.tile([128, NK, C], bf)
            nc.sync.dma_start(wk_sb, wk_r)
            wv_sb = p1w.tile([128, NK, C], bf)
            nc.sync.dma_start(wv_sb, wv_r)

            kv_ps = [kvps.tile([CH, 4, CH + 1], fp32, tag=f"kv{i}",
                               name=f"kvps{i}") for i in range(2)]

            for ci, (n0, sz) in enumerate(chunks):
                first, last = ci == 0, ci == len(chunks) - 1
                xh = p1roll.tile([128, NK, 128], bf, tag="xh")
                nc.sync.dma_start(xh[:, :, :sz], xT_r[:, :, n0:n0 + sz])

                v_sb = p1roll.tile([128, NH, CH + 1], bf, tag="v")
                nc.gpsimd.memset(v_sb[:, :, CH:CH + 1], 1.0)
                e_sb = p1roll.tile([128, NH, CH], bf, tag="e")
                for half in range(2):
                    c0 = half * 384
                    pv = p1ps.tile([128, 384], fp32, tag="pv")
                    pk = p1ps.tile([128, 384], fp32, tag="pk")
                    for k in range(NK):
                        nc.tensor.matmul(pv[:sz], xh[:, k, :sz],
                                         wv_sb[:, k, c0:c0 + 384],
                                         start=(k == 0), stop=(k == NK - 1))
                    for k in range(NK):
                        nc.tensor.matmul(pk[:sz], xh[:, k, :sz],
                                         wk_sb[:, k, c0:c0 + 384],
                                         start=(k == 0), stop=(k == NK - 1))
                    pv3 = pv[:sz].rearrange("p (a b) -> p a b", a=4)
                    pk3 = pk[:sz].rearrange("p (a b) -> p a b", a=4)
                    nc.scalar.copy(v_sb[:sz, 4 * half:4 * half + 4, 0:CH], pv3)
                    nc.scalar.activation(e_sb[:sz, 4 * half:4 * half + 4, :],
                                         pk3, ACT.Exp)

                for h in range(NH):
                    kvp = kv_ps[h // 4]
                    nc.tensor.matmul(kvp[:, h % 4, :],
                                     e_sb[:sz, h, :],
                                     v_sb[:sz, h, :],
                                     start=first, stop=last,
                                     skip_group_check=True)

                if not first:  # transpose image tokens into vt (+ twin)
                    j0 = n0 - 1
                    for h in range(NH):
                        tp = p1ps.tile([CH, 128], bf, tag="tp")
                        nc.tensor.transpose(tp[:, :sz], v_sb[:sz, h, 0:CH],
                                            ident[:sz, :sz])
                        dst = vt[:, h, VOFF + j0:VOFF + j0 + sz]
                        nc.scalar.copy(dst, tp[:, :sz])

                if not first:
                    pump_conv(2, img_chunk=ci - 1)

            # finalize kv: kv = kv_aug[:, :96] * (scale/den) + scale*bv
            for h in range(NH):
                kvp = kv_ps[h // 4]
                nc.vector.reciprocal(r_sb[:, h:h + 1], kvp[:, h % 4, CH:CH + 1])
                nc.vector.tensor_scalar(kv_sb[:, h, :], kvp[:, h % 4, 0:CH],
                                        r_sb[:, h:h + 1], float(SCALE),
                                        op0=AL.mult, op1=AL.mult)
                nc.vector.tensor_tensor(kv_sb[:, h, :], bvs_sb[:, h, :],
                                        kv_sb[:, h, :], op=AL.add)

        # ---------------- pass 2: per-group q + factor + conv + att + proj
        with tc.tile_pool(name="p4roll", bufs=4) as p4roll, \
             tc.tile_pool(name="p4att", bufs=2) as p4att, \
             tc.tile_pool(name="p4xg", bufs=2) as p4xg, \
             tc.tile_pool(name="p4ps", bufs=2, space="PSUM") as p4ps:

            # cls column (token 0): factor_att only, crpe = 0
            xc = p4xg.tile([128, NK, GC], bf, tag="xg")
            nc.sync.dma_start(xc[:, :, 0:1], xT_r[:, :, 0:1])
            pqc = p4ps.tile([CH, NH], fp32, tag="pq")
            for h in range(NH):
                for k in range(NK):
                    nc.tensor.matmul(pqc[:, h:h + 1],
                                     wq_sb[:, k, h * CH:(h + 1) * CH],
                                     xc[:, k, 0:1], start=(k == 0),
                                     stop=(k == NK - 1), skip_group_check=True)
            qtc = p4roll.tile([CH, NH], bf, tag="qtc")
            nc.scalar.copy(qtc, pqc)
            nc.vector.tensor_tensor(qtc, bq_sb, qtc, op=AL.add)
            pfc = p4ps.tile([CH, NH], fp32, tag="pf")
            for h in range(NH):
                nc.tensor.matmul(pfc[:, h:h + 1], kv_sb[:, h, :],
                                 qtc[:, h:h + 1], start=True, stop=True,
                                 skip_group_check=True)
            atc = p4roll.tile([CH, NH], bf, tag="atc")
            nc.scalar.copy(atc, pfc)
            poc = p4ps.tile([128, GC], fp32, tag="po")
            for e in range(6):
                for h in range(NH):
                    nc.tensor.matmul(poc[:, e:e + 1],
                                     pw_sb[:, h, e * 128:(e + 1) * 128],
                                     atc[:, h:h + 1], start=(h == 0),
                                     stop=(h == NH - 1), skip_group_check=True)
            ocs = p4roll.tile([128, 6], fp32, tag="ocs")
            for e in range(6):
                nc.scalar.activation(ocs[:, e:e + 1], poc[:, e:e + 1],
                                     ACT.Identity, bias=pb_sb[:, e:e + 1])
                nc.sync.dma_start(out_d[e * 128:(e + 1) * 128, 0:1],
                                  ocs[:, e:e + 1])

            # main grouped loop. Per-head chain PE(q)->ACT(qt)->PE(pf,conv)
            # ->DVE(ev,att) is software-pipelined: q runs one head ahead.
            vt3 = [vt[:, h, :].rearrange("p (y x) -> p y x", x=W)
                   for h in range(NH)]

            def emit_q(xg, h):
                pq = p4ps.tile([CH, GC], fp32, tag="pq", name="pq")
                for k in range(NK):
                    nc.tensor.matmul(pq, wq_sb[:, k, h * CH:(h + 1) * CH],
                                     xg[:, k, :], start=(k == 0),
                                     stop=(k == NK - 1))
                qt = p4roll.tile([CH, GC], bf, tag="qt", name="qt")
                nc.scalar.activation(qt, pq, ACT.Identity,
                                     bias=bq_sb[:, h:h + 1])
                return qt

            for g in range(GROUPS):
                gy0 = g * 8
                n0 = 1 + g * GC
                xg = p4xg.tile([128, NK, GC], bf, tag="xg")
                nc.sync.dma_start(xg, xT_r[:, :, n0:n0 + GC])
                drain_conv(g // 2)  # acc tiles this group reads must exist

                att = p4att.tile([CH, NH, GC], bf, tag="att")
                qt = emit_q(xg, 0)
                for h in range(NH):
                    qt_next = emit_q(xg, h + 1) if h < NH - 1 else None
                    on_dve = h in K7 and g < DVE_G[h]

                    # factor_att
                    pf = p4ps.tile([CH, GC], fp32, tag="pf")
                    nc.tensor.matmul(pf, kv_sb[:, h, :], qt,
                                     start=True, stop=True)

                    ev = p4roll.tile([CH, GC], bf, tag="ev")
                    if on_dve:
                        bi, half = g // 2, g % 2
                        acc = acc_t[(h, bi)][:, half * GC:half * GC + GC]
                        # ev = (acc + cb) * qt
                        nc.vector.scalar_tensor_tensor(
                            ev, acc, cb_sb[:, h:h + 1], qt,
                            op0=AL.add, op1=AL.mult)
                    else:
                        # conv: per-tap diagonal matmuls accumulating in psum
                        pcv = p4ps.tile([CH, 8, W], fp32, tag="pcv")
                        t_base = TAP_OFF[h]
                        for t, (dy, dx) in enumerate(TAPS[h]):
                            x0, x1 = max(0, -dx), W - max(0, dx)
                            ylo, yhi = gy0 + dy + YPAD, gy0 + 8 + dy + YPAD
                            nc.tensor.matmul(
                                pcv[:, 0:8, x0:x1], dg_sb[:, t_base + t, :],
                                vt3[h][:, ylo:yhi, x0 + dx:x1 + dx],
                                start=(t == 0), stop=(t == len(TAPS[h]) - 1),
                                skip_group_check=True)
                        # ev = (pcv + cb) * qt, straight from PSUM
                        nc.vector.scalar_tensor_tensor(
                            ev, pcv.rearrange("p a b -> p (a b)"),
                            cb_sb[:, h:h + 1], qt, op0=AL.add, op1=AL.mult)
                    # att = ev + factor (reads pf straight from PSUM)
                    nc.vector.scalar_tensor_tensor(att[:, h, :], ev, 1.0, pf,
                                                   op0=AL.mult, op1=AL.add)
                    qt = qt_next
                    pump_conv(1)

                # proj for this group of columns
                for e in range(6):
                    po = p4ps.tile([128, GC], fp32, tag="po")
                    for h in range(NH):
                        nc.tensor.matmul(po, pw_sb[:, h, e * 128:(e + 1) * 128],
                                         att[:, h, :], start=(h == 0),
                                         stop=(h == NH - 1))
                    osb = p4roll.tile([128, GC], fp32, tag="osb")
                    nc.scalar.activation(osb, po, ACT.Identity,
                                         bias=pb_sb[:, e:e + 1])
                    nc.sync.dma_start(out_d[e * 128:(e + 1) * 128, n0:n0 + GC],
                                      osb)

    nc.compile()
    return nc


def _get_program():
    global _PROG
    if _PROG is None:
        _PROG = _build_program()
    return _PROG


def _host_prep(x, qkv_w, qkv_b, proj_w, proj_b,
               conv3_w, conv3_b, conv5_w, conv5_b, conv7_w, conv7_b):
    """Build per-core input dicts (shared weight tensors prepped once)."""
    qkv_w = np.asarray(qkv_w, np.float32)
    qkv_b = np.asarray(qkv_b, np.float32)
    proj_w = np.asarray(proj_w, np.float32)
    proj_b = np.asarray(proj_b, np.float32)

    def wslab(sl):
        return np.ascontiguousarray(qkv_w[sl].T).astype(bf16)

    wq = wslab(slice(0, C))
    wk = wslab(slice(C, 2 * C))
    wv = wslab(slice(2 * C, 3 * C))
    bq96 = np.ascontiguousarray(qkv_b[0:C].reshape(NH, CH).T).astype(np.float32)
    bv = qkv_b[2 * C:3 * C].reshape(NH, CH)  # [h, c]

    pw96 = np.ascontiguousarray(
        proj_w.T.reshape(NH, CH, C).transpose(1, 0, 2)).astype(bf16)
    pb2 = np.ascontiguousarray(proj_b.reshape(6, 128).T).astype(np.float32)

    conv_w = [np.asarray(w, np.float32) for w in (conv3_w, conv5_w, conv7_w)]
    conv_b = [np.asarray(b, np.float32) for b in (conv3_b, conv5_b, conv7_b)]
    grp_of_head = [0, 0, 1, 1, 1, 2, 2, 2]
    head_in_grp = [0, 1, 0, 1, 2, 0, 1, 2]

    cb96 = np.zeros((CH, NH), np.float32)
    dg = np.zeros((CH, NTAP, CH), np.float32)
    cw = np.zeros((CH, NH, 49), np.float32)
    bvs = np.zeros((CH, NH, CH), np.float32)
    for h in range(NH):
        k = HEAD_KS[h]
        p = k // 2
        gidx, hig = grp_of_head[h], head_in_grp[h]
        wfull = conv_w[gidx][hig * CH:(hig + 1) * CH, 0]  # [96, k, k]
        # conv bias + folded v-bias: conv(v0+bv) = conv(v0) + sum(w)*bv
        cb96[:, h] = (conv_b[gidx][hig * CH:(hig + 1) * CH]
                      + wfull.sum(axis=(1, 2)) * bv[h])
        bvs[:, h, :] = SCALE * bv[h][None, :]
        for t, (dy, dx) in enumerate(TAPS[h]):
            np.fill_diagonal(dg[:, TAP_OFF[h] + t, :], wfull[:, dy + p, dx + p])
            cw[:, h, t] = wfull[:, dy + p, dx + p]
    dg = dg.astype(bf16)

    shared = {"wq": wq, "wk": wk, "wv": wv, "pw96": pw96, "pb2": pb2,
              "cb96": cb96, "bq96": bq96, "bvs": bvs.astype(bf16),
              "dg": dg, "cw": cw}

    x = np.asarray(x, np.float32)
    in_maps = []
    for b in range(B):
        m = dict(shared)
        m["xT"] = np.ascontiguousarray(x[b].T).astype(bf16)
        in_maps.append(m)
    return in_maps


def kernel(x, qkv_w, qkv_b, proj_w, proj_b,
           conv3_w, conv3_b, conv5_w, conv5_b, conv7_w, conv7_b, H, W,
           _trace=False):
    assert int(H) == 56 and int(W) == 56
    x = np.asarray(x)
    assert x.shape == (B, N, C)

    from concourse.bass_utils import run_bass_kernel_spmd
    nc = _get_program()
    in_maps = _host_prep(x, qkv_w, qkv_b, proj_w, proj_b,
                         conv3_w, conv3_b, conv5_w, conv5_b, conv7_w, conv7_b)
    res = run_bass_kernel_spmd(nc, in_maps, core_ids=list(range(B)), trace=_trace)
    out = np.stack([res.results[b]["outT"].T for b in range(B)])
    if _trace:
        kernel._last_results = res
    return out.astype(np.float32)
